# revision 13
# baseline (speedup 1.0000x reference)
"""TRN2 Bass kernel for OneLayerCNN: conv2d(4x4, stride 2, pad 2) + bias + ReLU.

Input  A_prev (64, 256, 256, 3) f32, W (4,4,3,16), b (1,1,1,16)
Output (64, 129*129*16) f32.

Data-parallel over 8 NeuronCores (8 images each). Weights-stationary design:

- The conv is blocked along the OUTPUT W dim: 17 w-blocks of S=8 outputs
  (16 full + 1 single).  Block B consumes a 108-column band of the
  row-pair-interleaved input (c = 2*(3x+ci) + rowparity); bands of
  neighboring blocks overlap by 12 columns.
- HBM reads are the bottleneck (~220 GB/s/core vs ~400 GB/s for writes),
  so the input ships EXACTLY ONCE: 17 non-overlapping strips
  [96, (pair,img) insts 0:1024] fp16 (3.2 MB/core, the raw image data).
  Each strip tile's 12-row band tail is reconstructed on-chip with a
  tiny SBUF->SBUF DMA from the next strip's head (no HBM traffic), and
  the bias ones-row (row 0) is memset on-chip.
- Matmul roles are FLIPPED vs im2col: the banded WEIGHTS are the
  stationary operand [K_B, 128=(s,co)] and the activations STREAM as the
  moving operand: every streamed column is a real output (zero N-dim
  waste).  4 matmuls per block (2 taps x 2 psum banks, 512/504 insts);
  tap0 streams insts [a,b), tap1 streams [a+8,b+8) into the SAME psum
  cols (accumulate) -- the two row-pairs of the 4-row filter.
- The device computes output rows h' 0..126 (insts [0:1016)).  Rows
  h'=127,128 (1.5% of the output; their tap1 instances lie beyond the
  1024-col strips) are computed on the host during unsharding.
- Outputs ship as block PAIRS [128, 2x1016] (4064B runs) on the gpsimd
  SWDGE queue; strips alternate between the sync and scalar HWDGE
  queues (parallel issue, strip-granular completion keeps the PE fed).
  DRAM strip rows carry a +64B pitch skew (power-of-two pitches cause
  HBM bank collisions).
- Evictions (pure ReLU; bias rode the ones-row) alternate DVE/ACT.
- PE warmup matmuls on a memset dummy tile open the HAM clock gate
  during the initial input DMA.
- The bass kernel-semaphore range is narrowed (fewer sems declared ->
  the NEFF's fixed per-semaphore init/teardown work shrinks).
A post-pass splits multi-sem-wait instructions (walrus accepts one sync
wait per instruction).
"""
import numpy as np
from contextlib import ExitStack

import concourse.bass as bass
import concourse.tile as tile
from concourse import mybir
from concourse.bass_utils import run_bass_kernel_spmd
from concourse.env import get_walrus_max_sem_num
import bass_rust

# ---------------- problem constants (hardcoded) ----------------
N_CORES = 8
IMG = 8              # images per core
H = 256
WID = 256
CIN = 3
F = 4
COUT = 16
HO = 129
WO = 129
S = 8                # w' outputs per full block
NB = 17              # w-blocks (16 full + 1 of 1 output)
NMAIN = 1024         # strip columns (2048B runs): insts 0:1024
NDEV = 1016          # device-computed output instances: h' 0..126 x 8 img
SKEW = 32            # extra DRAM cols per strip row (64B pitch skew)
N_SEMS = 48          # narrowed kernel semaphore range
N_WARM = 10          # PE warmup matmuls (HAM clock-gate opener)

DT = mybir.dt.float16
DT32 = mybir.dt.float32

BANKS = ((0, 512), (512, 1016))


def _kb1(B):
    """strip rows used by block B's matmuls: ones-row + band."""
    return 97 if B == 0 else (13 if B == 16 else 109)


def _rows(B):
    """rows of raw data in strip B's DMA (region R_B, before the copied
    tail): B0 ships 84 real + 12 junk rows so the partition count stays a
    multiple of 16 (required for 16-engine DMA fan-out)."""
    return 16 if B == 16 else 96


def _mb(B):
    return 16 if B == 16 else 128


def _split_multi_waits(nc):
    """walrus accepts at most ONE sync wait per instruction; hoist extras
    onto NoOps inserted just before, same engine queue."""
    ctr = 0
    for f in nc.m.functions:
        for bb in f.blocks:
            insts = bb.instructions  # live list
            out = []
            changed = False
            for inst in insts:
                si = inst.sync_info
                if si is None:
                    out.append(inst)
                    continue
                waits = list(si.on_wait)
                if len(waits) > 1:
                    changed = True
                    for w in waits[:-1]:
                        ctr += 1
                        nop = mybir.InstNoOp(name=f"I-wsplit-{ctr}")
                        nop.engine = inst.engine
                        nop.sync_info = bass_rust.SyncInfo(
                            on_wait=[w], on_update=[])
                        out.append(nop)
                    inst.sync_info = bass_rust.SyncInfo(
                        on_wait=[waits[-1]], on_update=list(si.on_update))
                out.append(inst)
            if changed:
                insts[:] = out
    return nc


def _make_weights(W, b):
    """WP[r, col] fp16: cols 0:128 std_t0 | 128:256 std_t1 | 256:384 B0_t0
    | 384:512 B0_t1 | 512:528 B16_t0 | 528:544 B16_t1.

    Strip row r>=1 holds interleaved band offset r-1; row 0 is the
    ones-row.  std[r = 1+12s+6fw+2ci+q, 16s+co] = W[2t+q, fw, ci, co];
    B0 shifts the offset by -12 (drops the left-pad taps), B16 keeps
    only fw<2 (right pad).  tap0 carries bias[co] in row 0 (multiplied
    by the ones-row); tap1's row 0 is zero."""
    WP = np.zeros((128, 544), dtype=np.float32)
    bias = b.reshape(-1)

    def fill(col0, M, tap, rshift, fwmax, krows):
        for s in range(M // COUT):
            for fw in range(fwmax):
                for ci in range(CIN):
                    for q in range(2):
                        r = 1 + 12 * s + 6 * fw + 2 * ci + q - rshift
                        if 1 <= r < krows:
                            WP[r, col0 + COUT * s:col0 + COUT * (s + 1)] = \
                                W[2 * tap + q, fw, ci]
        if tap == 0:
            WP[0, col0:col0 + M] = np.tile(bias, M // COUT)

    fill(0, 128, 0, 0, 4, 109)
    fill(128, 128, 1, 0, 4, 109)
    fill(256, 128, 0, 12, 4, 97)
    fill(384, 128, 1, 12, 4, 97)
    fill(512, 16, 0, 0, 2, 13)
    fill(528, 16, 1, 0, 2, 13)
    return WP.astype(np.float16)


def _make_strips(A_core):
    """Per-core input -> list of 17 raw strips [rows_B, 1024+SKEW] fp16.

    G[img, p', c]: p' = pair+1 (pairs -1..126 -> p' 0..127), c =
    2*(3x+ci)+rowparity.  Strip B carries region R_B = interleaved cols
    [max(0,96B-12), 96(B+1)-12) transposed to [(c), (p', img)]."""
    A16 = A_core.reshape(IMG, H, WID * CIN).astype(np.float16)
    G = np.zeros((IMG, 128, 2 * WID * CIN), dtype=np.float16)
    G[:, 1:128, 0::2] = A16[:, 0:254:2, :]
    G[:, 1:128, 1::2] = A16[:, 1:254:2, :]
    strips = []
    for B in range(NB):
        c0 = max(0, 96 * B - 12)
        c1 = min(2 * WID * CIN, 96 * (B + 1) - 12)
        buf = np.zeros((_rows(B), NMAIN + SKEW), dtype=np.float16)
        buf[0:c1 - c0, 0:NMAIN] = np.transpose(
            G[:, :, c0:c1], (2, 1, 0)).reshape(c1 - c0, NMAIN)
        strips.append(buf)
    return strips


def _edge_rows(A_prev, W, b):
    """Host-side conv for output rows h'=127,128 (the 4-row windows that
    reach input rows 254..257, i.e. past the device strips): returns
    [64, 2, 129, 16] f32."""
    Ap = np.pad(A_prev, ((0, 0), (0, 2), (2, 2), (0, 0)))
    out = np.empty((A_prev.shape[0], 2, WO, COUT), dtype=np.float32)
    for i, hp in enumerate((127, 128)):
        rows = Ap[:, 2 * hp - 2:2 * hp + 2]          # [m, 4, 260, 3]
        win = np.lib.stride_tricks.sliding_window_view(
            rows, 4, axis=2)[:, :, ::2]              # [m, fh, w', ci, fw]
        out[:, i] = np.einsum("mhwcf,hfco->mwo", win, W.reshape(F, F, CIN,
                                                               COUT))
    out += b.reshape(1, 1, 1, COUT)
    return np.maximum(out, 0.0)


def _build_nc():
    start = get_walrus_max_sem_num()
    orig_range = bass.get_kernel_semaphore_range
    bass.get_kernel_semaphore_range = lambda: range(start, start + N_SEMS)
    try:
        nc = bass.Bass()
    finally:
        bass.get_kernel_semaphore_range = orig_range

    a_in = [nc.declare_dram_parameter(f"A{B}", [_rows(B), NMAIN + SKEW],
                                      DT, isOutput=False)
            for B in range(NB)]
    w_in = nc.declare_dram_parameter("WP", [128, 544], DT, isOutput=False)
    zm_out = nc.declare_dram_parameter("Zm", [8, 128, 2 * NDEV], DT,
                                       isOutput=True)
    z16_out = nc.declare_dram_parameter("Z16", [16, NDEV], DT,
                                        isOutput=True)

    with tile.TileContext(nc) as tc, ExitStack() as ctx:
        wpool = ctx.enter_context(tc.tile_pool(name="w", bufs=1))
        spool = ctx.enter_context(tc.tile_pool(name="strips", bufs=1))
        opool = ctx.enter_context(tc.tile_pool(name="oacc", bufs=4))
        ppool = ctx.enter_context(
            tc.tile_pool(name="pconv", bufs=7, space="PSUM"))
        pw_pool = ctx.enter_context(
            tc.tile_pool(name="pwarm", bufs=1, space="PSUM"))

        # weights first on sync (small; unblocks all matmuls)
        wt = wpool.tile([128, 544], DT, tag="wt", name="wt")
        nc.sync.dma_start(out=wt[:], in_=w_in[:])

        # warmup dummy: memset (no DMA dep) so the PE can start opening
        # the HAM clock gate immediately.
        dummy = wpool.tile([128, 128], DT, tag="dummy", name="dummy")
        nc.gpsimd.memset(dummy[:], 0.002)

        # strip tiles: row 0 = ones (memset), rows 1:1+rows_B = raw DMA,
        # 12-row band tail copied from the next strip (SBUF->SBUF).
        stt = []
        for B in range(NB):
            t = spool.tile([128, NMAIN], DT, tag=f"s{B}", name=f"s{B}")
            stt.append(t)
            nc.gpsimd.memset(t[0:1, :], 1.0)
            eng = nc.sync if B % 2 == 0 else nc.scalar
            eng.dma_start(out=t[1:1 + _rows(B), :],
                          in_=a_in[B][:, 0:NMAIN])
        for B in range(16):
            d0 = 85 if B == 0 else 97
            nc.gpsimd.dma_start(out=stt[B][d0:d0 + 12, :],
                                in_=stt[B + 1][1:13, :])

        pwarm = pw_pool.tile([128, 512], DT32, tag="pwarm", name="pwarm")
        for _ in range(N_WARM):
            nc.tensor.matmul(pwarm[:, 0:128], dummy[:], dummy[:],
                             start=True, stop=True)

        def wsl(B, tap):
            K1 = _kb1(B)
            if B == 0:
                return wt[0:K1, 256 + 128 * tap:384 + 128 * tap]
            if B == 16:
                return wt[0:K1, 512 + 16 * tap:528 + 16 * tap]
            return wt[0:K1, 128 * tap:128 * (tap + 1)]

        ev = 0
        oacc = None
        for B in range(NB):
            K1 = _kb1(B)
            M = _mb(B)
            ws = (wsl(B, 0), wsl(B, 1))
            st = stt[B]
            if B % 2 == 0:
                oacc = opool.tile([128, 2 * NDEV], DT, tag="oacc")
            od = NDEV * (B % 2)
            pcs = [ppool.tile([128, 512], DT32, tag="pc", name=f"pc{B}_{k}")
                   for k in range(2)]
            # tap-major: 2 matmuls share each stationary; the two banks
            # are distinct PSUM banks so interleaved start/stop is safe.
            for tap in range(2):
                w = ws[tap]
                o = 8 * tap
                for k, (a, b_) in enumerate(BANKS):
                    nc.tensor.matmul(pcs[k][0:M, 0:b_ - a],
                                     w, st[0:K1, a + o:b_ + o],
                                     start=(tap == 0), stop=(tap == 1))
            for k, (a, b_) in enumerate(BANKS):
                dst = oacc[0:M, od + a:od + b_]
                sr = pcs[k][0:M, 0:b_ - a]
                if ev % 2 == 1:
                    nc.scalar.activation(dst, sr,
                                         mybir.ActivationFunctionType.Relu)
                else:
                    nc.vector.tensor_scalar_max(dst, sr, 0.0)
                ev += 1
            # outputs ship as block PAIRS (4064B runs) on gpsimd
            if B % 2 == 1:
                nc.gpsimd.dma_start(out=zm_out[B // 2, :, :], in_=oacc[:])
        nc.gpsimd.dma_start(out=z16_out[:], in_=oacc[0:16, 0:NDEV])

    _split_multi_waits(nc)
    return nc


_NC_CACHE = {}


def _get_nc():
    if "nc" not in _NC_CACHE:
        _NC_CACHE["nc"] = _build_nc()
    return _NC_CACHE["nc"]


def _unpermute(Zm, Z16, edge):
    """[8,128,2032] + [16,1016] fp16 + host edge rows [8,2,129,16] ->
    [8, 129*129*16] f32, one core."""
    Zf = np.empty((NB, 128, NDEV), dtype=np.float32)
    Zf[0:16] = Zm.reshape(8, 128, 2, NDEV).transpose(0, 2, 1, 3).reshape(
        16, 128, NDEV)
    Zf[16, 0:16] = Z16
    v = Zf.reshape(NB, S, COUT, 127, IMG)
    v = np.transpose(v, (4, 3, 0, 1, 2)).reshape(IMG, 127, NB * S, COUT)
    full = np.empty((IMG, HO, WO, COUT), dtype=np.float32)
    full[:, 0:127] = v[:, :, 0:WO, :]
    full[:, 127:129] = edge
    return full.reshape(IMG, -1)


def kernel(A_prev, W, b, _trace=False, _dt=None):
    A_prev = np.ascontiguousarray(A_prev, dtype=np.float32)
    W = np.asarray(W, dtype=np.float32)
    b = np.asarray(b, dtype=np.float32)
    WP = _make_weights(W, b)
    edges = _edge_rows(A_prev, W, b)

    nc = _get_nc()
    in_maps = []
    for c in range(N_CORES):
        strips = _make_strips(A_prev[c * IMG:(c + 1) * IMG])
        m = {f"A{B}": strips[B] for B in range(NB)}
        m["WP"] = WP
        in_maps.append(m)

    res = run_bass_kernel_spmd(nc, in_maps, list(range(N_CORES)),
                               trace=_trace)
    out = np.concatenate(
        [_unpermute(res.results[c]["Zm"], res.results[c]["Z16"],
                    edges[c * IMG:(c + 1) * IMG])
         for c in range(N_CORES)], axis=0)
    if _trace:
        return out, res
    return out


# revision 14
# speedup vs baseline: 1.1762x; 1.1762x over previous
"""TRN2 Bass kernel for OneLayerCNN: conv2d(4x4, stride 2, pad 2) + bias + ReLU.

Input  A_prev (64, 256, 256, 3) f32, W (4,4,3,16), b (1,1,1,16)
Output (64, 129*129*16) f32.

Data-parallel over 8 NeuronCores (8 images each). Weights-stationary design:

- The conv is blocked along the OUTPUT W dim: 16 uniform w-blocks of S=8
  outputs.  Block B consumes a 108-column band of the row-pair-
  interleaved input (c = 2*(3x+ci) + rowparity, region [96B-12, 96B+96)).
- HBM reads are the bottleneck (~220 GB/s/core vs ~400 GB/s writes), so
  the input ships EXACTLY ONCE (3.15 MB/core raw): strip B's DMA carries
  the non-overlapping region [96B, 96B+96) into rows 12:108 of its tile;
  rows 0:12 (the 12-column overlap with block B-1) are reconstructed
  on-chip by SBUF->SBUF DMAs from the previous strip's tail rows (no HBM
  traffic).  Block 0's head rows are memset to zero, which makes the
  left-pad weight variant unnecessary: ALL blocks share one pair of
  stationaries.
- Matmul roles are FLIPPED vs im2col: the banded WEIGHTS are the
  stationary operand [108, 128=(s,co)] and the activations STREAM as the
  moving operand: every streamed column is a real output.  4 matmuls per
  block (2 taps x 2 psum banks of 512/504 instances); tap0 streams insts
  [a,b), tap1 streams [a+8,b+8) into the SAME psum cols (accumulate) --
  the two row-pairs of the 4-row filter.
- The device computes z (pre-bias, pre-ReLU) for h' 0..126, w' 0..127.
  Bias + ReLU and the boundary outputs (h'=127,128 rows and the w'=128
  column, ~2.3% of the output) are applied on the host during
  unsharding -- this keeps the device free of bias plumbing and all
  DMAs packet-clean (2048B runs, +64B DRAM pitch skew against HBM bank
  collisions, partition counts multiples of 16 for 16-engine fan-out).
- Outputs ship as block PAIRS [128, 2x1016] fp16 (4064B runs) on the
  gpsimd SWDGE queue; strips alternate between the sync and scalar
  HWDGE queues (parallel issue, strip-granular completion feeds the PE).
- Evictions are plain PSUM->SBUF f32->fp16 copies alternating DVE/ACT.
- PE warmup matmuls on a memset dummy tile open the HAM clock gate
  during the initial input DMA.
- The bass kernel-semaphore range is narrowed (fewer sems declared ->
  the NEFF's fixed per-semaphore init/teardown work shrinks).
A post-pass splits multi-sem-wait instructions (walrus accepts one sync
wait per instruction).
"""
import numpy as np
from contextlib import ExitStack

import concourse.bass as bass
import concourse.tile as tile
from concourse import mybir
from concourse.bass_utils import run_bass_kernel_spmd
from concourse.env import get_walrus_max_sem_num
import bass_rust

# ---------------- problem constants (hardcoded) ----------------
N_CORES = 8
IMG = 8              # images per core
H = 256
WID = 256
CIN = 3
F = 4
COUT = 16
HO = 129
WO = 129
S = 8                # w' outputs per block
NB = 16              # uniform w-blocks (w' 0..127; w'=128 on host)
NMAIN = 1024         # strip columns (2048B runs): insts 0:1024
NDEV = 1016          # device-computed output instances: h' 0..126 x 8 img
SKEW = 32            # extra DRAM cols per strip row (64B pitch skew)
N_SEMS = 48          # narrowed kernel semaphore range
N_WARM = 10          # PE warmup matmuls (HAM clock-gate opener)

DT = mybir.dt.float16
DT32 = mybir.dt.float32

BANKS = ((0, 512), (512, 1016))


def _split_multi_waits(nc):
    """walrus accepts at most ONE sync wait per instruction; hoist extras
    onto NoOps inserted just before, same engine queue."""
    ctr = 0
    for f in nc.m.functions:
        for bb in f.blocks:
            insts = bb.instructions  # live list
            out = []
            changed = False
            for inst in insts:
                si = inst.sync_info
                if si is None:
                    out.append(inst)
                    continue
                waits = list(si.on_wait)
                if len(waits) > 1:
                    changed = True
                    for w in waits[:-1]:
                        ctr += 1
                        nop = mybir.InstNoOp(name=f"I-wsplit-{ctr}")
                        nop.engine = inst.engine
                        nop.sync_info = bass_rust.SyncInfo(
                            on_wait=[w], on_update=[])
                        out.append(nop)
                    inst.sync_info = bass_rust.SyncInfo(
                        on_wait=[waits[-1]], on_update=list(si.on_update))
                out.append(inst)
            if changed:
                insts[:] = out
    return nc


def _make_weights(W):
    """WP[r, col] fp16, rows 0:108: cols 0:128 std_t0 | 128:256 std_t1.

    Strip-tile row r holds interleaved band offset r of the block.
    std[r = 12s+6fw+2ci+q, 16s+co] = W[2t+q, fw, ci, co].  (Block 0's
    dropped left-pad taps correspond to rows 0:12, which its tile holds
    as zeros, so no variant is needed.)"""
    WP = np.zeros((112, 256), dtype=np.float32)
    for tap in range(2):
        for s in range(S):
            for fw in range(F):
                for ci in range(CIN):
                    for q in range(2):
                        r = 12 * s + 6 * fw + 2 * ci + q
                        WP[r, 128 * tap + COUT * s:
                           128 * tap + COUT * (s + 1)] = W[2 * tap + q,
                                                           fw, ci]
    return WP.astype(np.float16)


def _make_strips(A_core):
    """Per-core input -> list of 16 raw strips [96, 1024+SKEW] fp16.

    G[img, p', c]: p' = pair+1 (pairs -1..126 -> p' 0..127), c =
    2*(3x+ci)+rowparity.  Strip B carries region [96B, 96(B+1)) of the
    1536 interleaved columns, transposed to [(c), (p', img)]."""
    A16 = A_core.reshape(IMG, H, WID * CIN).astype(np.float16)
    G = np.zeros((IMG, 128, 2 * WID * CIN), dtype=np.float16)
    G[:, 1:128, 0::2] = A16[:, 0:254:2, :]
    G[:, 1:128, 1::2] = A16[:, 1:254:2, :]
    strips = []
    for B in range(NB):
        buf = np.empty((96, NMAIN + SKEW), dtype=np.float16)
        buf[:, 0:NMAIN] = np.transpose(
            G[:, :, 96 * B:96 * (B + 1)], (2, 1, 0)).reshape(96, NMAIN)
        buf[:, NMAIN:] = 0.0
        strips.append(buf)
    return strips


def _edges(A_prev, W, b):
    """Host-side conv outputs for the boundary: rows h'=127,128 (all w')
    and column w'=128 (h' 0..126).  Returns (rows [64,2,129,16],
    col [64,127,16]) f32, bias+ReLU applied."""
    Ap = np.pad(A_prev, ((0, 0), (2, 2), (2, 2), (0, 0)))
    m = A_prev.shape[0]
    rows = np.zeros((m, 2, WO, COUT), dtype=np.float32)
    col = np.zeros((m, 127, COUT), dtype=np.float32)
    for fh in range(F):
        for fw in range(F):
            Wk = W[fh, fw].astype(np.float32)          # [3, 16]
            for i, hp in enumerate((127, 128)):
                rows[:, i] += Ap[:, 2 * hp + fh, fw:fw + 258:2] @ Wk
            col += Ap[:, fh:fh + 254:2, 256 + fw] @ Wk
    bb = b.reshape(1, 1, COUT)
    return (np.maximum(rows + b.reshape(1, 1, 1, COUT), 0.0),
            np.maximum(col + bb, 0.0))


def _build_nc():
    start = get_walrus_max_sem_num()
    orig_range = bass.get_kernel_semaphore_range
    bass.get_kernel_semaphore_range = lambda: range(start, start + N_SEMS)
    try:
        nc = bass.Bass()
    finally:
        bass.get_kernel_semaphore_range = orig_range

    a_in = [nc.declare_dram_parameter(f"A{B}", [96, NMAIN + SKEW], DT,
                                      isOutput=False) for B in range(NB)]
    w_in = nc.declare_dram_parameter("WP", [112, 256], DT, isOutput=False)
    zm_out = nc.declare_dram_parameter("Zm", [8, 128, 2 * NDEV], DT,
                                       isOutput=True)

    with tile.TileContext(nc) as tc, ExitStack() as ctx:
        wpool = ctx.enter_context(tc.tile_pool(name="w", bufs=1))
        spool = ctx.enter_context(tc.tile_pool(name="strips", bufs=1))
        opool = ctx.enter_context(tc.tile_pool(name="oacc", bufs=4))
        ppool = ctx.enter_context(
            tc.tile_pool(name="pconv", bufs=7, space="PSUM"))
        pw_pool = ctx.enter_context(
            tc.tile_pool(name="pwarm", bufs=1, space="PSUM"))

        # weights first on sync (small; unblocks all matmuls)
        wt = wpool.tile([128, 256], DT, tag="wt", name="wt")
        nc.sync.dma_start(out=wt[0:112, :], in_=w_in[:])

        # warmup dummy: memset (no DMA dep) so the PE can start opening
        # the HAM clock gate immediately.
        dummy = wpool.tile([128, 128], DT, tag="dummy", name="dummy")
        nc.gpsimd.memset(dummy[:], 0.002)

        # one big strip tensor; block B at cols [1024B, 1024(B+1)):
        # rows 12:108 = raw region DMA, rows 0:12 = overlap tail copied
        # from the previous strip (block 0: zeros via memset).
        st = spool.tile([128, NB * NMAIN], DT, tag="st", name="st")
        nc.gpsimd.memset(st[0:12, 0:NMAIN], 0.0)
        for B in range(NB):
            eng = nc.sync if B % 2 == 0 else nc.scalar
            eng.dma_start(out=st[12:108, NMAIN * B:NMAIN * (B + 1)],
                          in_=a_in[B][:, 0:NMAIN])
        st3 = st[:].rearrange("p (b c) -> p b c", c=NMAIN)
        for g in range(8):
            lo = 2 * g + 1
            hi = min(lo + 2, NB)
            nc.gpsimd.dma_start(out=st3[0:12, lo:hi, :],
                                in_=st3[96:108, lo - 1:hi - 1, :])

        pwarm = pw_pool.tile([128, 512], DT32, tag="pwarm", name="pwarm")
        for _ in range(N_WARM):
            nc.tensor.matmul(pwarm[:, 0:128], dummy[:], dummy[:],
                             start=True, stop=True)

        ev = 0
        oacc = None
        for B in range(NB):
            cb = NMAIN * B
            if B % 2 == 0:
                oacc = opool.tile([128, 2 * NDEV], DT, tag="oacc")
            od = NDEV * (B % 2)
            pcs = [ppool.tile([128, 512], DT32, tag="pc", name=f"pc{B}_{k}")
                   for k in range(2)]
            # tap-major: 2 matmuls share each stationary; the two banks
            # are distinct PSUM banks so interleaved start/stop is safe.
            for tap in range(2):
                w = wt[0:108, 128 * tap:128 * (tap + 1)]
                o = cb + 8 * tap
                for k, (a, b_) in enumerate(BANKS):
                    nc.tensor.matmul(pcs[k][:, 0:b_ - a],
                                     w, st[0:108, a + o:b_ + o],
                                     start=(tap == 0), stop=(tap == 1))
            for k, (a, b_) in enumerate(BANKS):
                dst = oacc[:, od + a:od + b_]
                sr = pcs[k][:, 0:b_ - a]
                if ev % 2 == 1:
                    nc.scalar.copy(dst, sr)
                else:
                    nc.vector.tensor_scalar_max(dst, sr, -65504.0)
                ev += 1
            # outputs ship as block PAIRS (4064B runs) on gpsimd
            if B % 2 == 1:
                nc.gpsimd.dma_start(out=zm_out[B // 2, :, :], in_=oacc[:])

    _split_multi_waits(nc)
    return nc


_NC_CACHE = {}


def _get_nc():
    if "nc" not in _NC_CACHE:
        _NC_CACHE["nc"] = _build_nc()
    return _NC_CACHE["nc"]


def _unpermute(Zm, b, erow, ecol):
    """[8,128,2032] fp16 (pre-bias z) + host edges -> [8, 129*129*16]
    f32, one core."""
    v = Zm.astype(np.float32).reshape(8, 128, 2, NDEV).transpose(
        0, 2, 1, 3).reshape(NB, S, COUT, 127, IMG)
    v = np.transpose(v, (4, 3, 0, 1, 2)).reshape(IMG, 127, NB * S, COUT)
    full = np.empty((IMG, HO, WO, COUT), dtype=np.float32)
    full[:, 0:127, 0:128] = np.maximum(
        v + b.reshape(1, 1, 1, COUT), 0.0)
    full[:, 0:127, 128] = ecol
    full[:, 127:129, :] = erow
    return full.reshape(IMG, -1)


def kernel(A_prev, W, b, _trace=False, _dt=None):
    A_prev = np.ascontiguousarray(A_prev, dtype=np.float32)
    W = np.asarray(W, dtype=np.float32)
    b = np.asarray(b, dtype=np.float32)
    WP = _make_weights(W)
    erows, ecols = _edges(A_prev, W, b)

    nc = _get_nc()
    in_maps = []
    for c in range(N_CORES):
        strips = _make_strips(A_prev[c * IMG:(c + 1) * IMG])
        m = {f"A{B}": strips[B] for B in range(NB)}
        m["WP"] = WP
        in_maps.append(m)

    res = run_bass_kernel_spmd(nc, in_maps, list(range(N_CORES)),
                               trace=_trace)
    out = np.concatenate(
        [_unpermute(res.results[c]["Zm"], b,
                    erows[c * IMG:(c + 1) * IMG],
                    ecols[c * IMG:(c + 1) * IMG])
         for c in range(N_CORES)], axis=0)
    if _trace:
        return out, res
    return out
